# revision 57
# baseline (speedup 1.0000x reference)
"""EvaLinearAttention Trainium2 kernel.

Per-core math (one batch element per core, 8 cores, SPMD):
  qkv = x @ W_qkv.T + bias ; RoPE on q,k (interleaved pairs, prefix token 0
  identity) ; phi_* = softmax(*, -1)
  kv = phi_k.T @ v ; ksum = sum_n phi_k ; z = phi_q @ ksum
  attn = (phi_q @ kv) / (z + eps) ; out = LN(attn) @ proj_w.T + proj_b

Reformulation / schedule:
  - Phase 1 (one pass over x): token-major k/v per 128-token block with RoPE
    + exp; k-softmax denominator folded into v (vn = v/rowsum(exp_k));
    kv accumulated in bf16 at 130-free (128 kv cols + 2 ksum cols) across
    all 4096 tokens in 2 PSUM banks (3 pair-chains per bank). q computed
    feature-major per supertile, RoPE'd, exp'd (q softmax normalization
    cancels between attn numerator and z), stored bf16 in SBUF for phase 2.
  - Phase 2 (no HBM reads): per pair ya = kvext^T eq and z = ksum_bcast^T eq
    (block-diagonal broadcast ksum makes z come out already replicated to
    all 128 feature partitions, so no separate broadcast matmul); att =
    ya * 1/z. LayerNorm folded into the projection: pwg gets an extra ones
    column so sum_c(att) rides along in the proj PSUM; variance from a
    Square+ones-matmul chain; out = r*P + (r*mu)*(-wsum) + const applied
    as two fused scalar_tensor_tensor ops. Exp and Sqrt live in different
    activation tables; phase separation means 2 table loads total.
"""

import functools
import sys

import numpy as np

sys.path.insert(0, "/opt/trn_rl_repo")

import concourse.bass as bass
import concourse.bacc as bacc
import concourse.mybir as mybir
import concourse.tile as tile
from concourse.bass_utils import run_bass_kernel_spmd

B, N, C, H, D = 8, 4096, 768, 12, 64
NPT = 1
ST = 512            # tokens per super-tile
NST = N // ST       # 8
NB = ST // 128      # 4 sub-tiles of 128 tokens
NCT = C // 128      # 6 contraction tiles
NJ = NCT // 2       # 3 k-tile pairs for fp8 DoubleRow matmuls
NPAIR = H // 2      # 6 head pairs
EPS_LN = 1e-5
WS = 32.0           # fp8 weight pre-scale (avoids e4m3 denormals)
SU = 16.0           # v-path scale: smaller so fp8(att*SU) stays < 240
FP = mybir.dt.float32
FR = mybir.dt.float32r
BF = mybir.dt.bfloat16
F8 = mybir.dt.float8e4
DRM = mybir.MatmulPerfMode.DoubleRow

PAIRSWAP32 = [i ^ 1 for i in range(32)]


def _mm(nc, out, lhsT, rhs, start, stop):
    nc.tensor.matmul(out, lhsT.bitcast(FR), rhs.bitcast(FR), start=start,
                     stop=stop)


def _mmb(nc, out, lhsT, rhs, start, stop):
    nc.tensor.matmul(out, lhsT, rhs, start=start, stop=stop)


def build_nc(vb_zero=True):
    nc = bacc.Bacc("TRN2", target_bir_lowering=False)

    xpk8h = nc.dram_tensor("xpk8h", [NST, 128, NCT * ST], F8,
                           kind="ExternalInput")
    xpk8l = nc.dram_tensor("xpk8l", [NST, 128, NCT * ST], F8,
                           kind="ExternalInput")
    wq8h = nc.dram_tensor("wq8h", [NJ, 128, 2, C], F8, kind="ExternalInput")
    wq8l = nc.dram_tensor("wq8l", [NJ, 128, 2, C], F8, kind="ExternalInput")
    wkv8h = nc.dram_tensor("wkv8h", [NJ, 128, 2, 2 * C], F8,
                           kind="ExternalInput")
    wkv8l = nc.dram_tensor("wkv8l", [NJ, 128, 2, 2 * C], F8,
                           kind="ExternalInput")
    cosT2 = nc.dram_tensor("cosT2", [128, N], FP, kind="ExternalInput")
    sinT2 = nc.dram_tensor("sinT2", [128, N], FP, kind="ExternalInput")
    rope_pk = nc.dram_tensor("rope_pk", [NST, 128, NB, 2, D], FP,
                             kind="ExternalInput")
    qb = nc.dram_tensor("qb", [C], FP, kind="ExternalInput")
    vb = nc.dram_tensor("vb", [C], FP, kind="ExternalInput")
    pwgT = nc.dram_tensor("pwgT", [C, 770], FR, kind="ExternalInput")
    wsn2 = nc.dram_tensor("wsn2", [C], FP, kind="ExternalInput")
    constb = nc.dram_tensor("constb", [C], FP, kind="ExternalInput")
    out = nc.dram_tensor("out", [N, C], FP, kind="ExternalOutput")

    with tile.TileContext(nc) as tc:
        with (
            tc.tile_pool(name="common", bufs=1) as common,
            tc.tile_pool(name="xa", bufs=2) as xpool,
        ):
            # persistent tiles
            wqh = [common.tile([128, 2, C], F8, name=f"wqh{j}", tag=f"wqh{j}")
                   for j in range(NJ)]
            wql = [common.tile([128, 2, C], F8, name=f"wql{j}", tag=f"wql{j}")
                   for j in range(NJ)]
            pwgs = [common.tile([128, 770], FR, name=f"pwg{ct}",
                                tag=f"pwg{ct}") for ct in range(NCT)]
            vbb = common.tile([128, C], FP)
            qb_sb = common.tile([128, NCT], FP)
            cb_sb = common.tile([128, C], FP)
            wsb = common.tile([128, C], FP)
            one11 = common.tile([1, 1], FP)
            nc.vector.memset(one11, 1.0)
            # bf16 constants via f32 memset + converting copy (bf16 memset
            # fails the hw ISA check)
            onesf = common.tile([128, 1], FP)
            nc.vector.memset(onesf, 1.0)
            ones128b = common.tile([128, 1], BF)
            nc.vector.tensor_copy(ones128b, onesf)
            zf = common.tile([128, 3, 128], FP)
            nc.vector.memset(zf, 0.0)
            # per-bank tiles so phase 2's first matmuls only wait on their
            # own bank's assembly ops
            kvx2 = [common.tile([128, 3, 128], BF, name=f"kvx{i}",
                                tag=f"kvx{i}") for i in range(2)]
            ksdb = [common.tile([128, 3, 128], BF, name=f"ksd{i}",
                                tag=f"ksd{i}") for i in range(2)]
            for i in range(2):
                nc.vector.tensor_copy(kvx2[i], zf)
                nc.vector.tensor_copy(ksdb[i], zf)
            eqs = [common.tile([128, N], BF, name=f"eqs{fq}", tag=f"eqs{fq}")
                   for fq in range(NPAIR)]

            # qb is tiny and needed by the first q exp; the rest are emitted
            # after the weight loads
            nc.gpsimd.dma_start(
                out=qb_sb, in_=qb.rearrange("(a p) -> p a", p=128))

            # ============ PHASE 1: x -> (kv, ksum) accum + eq store ========
            with (
                tc.tile_pool(name="wkvp", bufs=1) as wkvp,
                tc.tile_pool(name="sa", bufs=2) as sa,
                tc.tile_pool(name="wk", bufs=2) as wk,
                tc.tile_pool(name="ekp", bufs=2) as ekpool,
                tc.tile_pool(name="vnxp", bufs=2) as vnxpool,
                tc.tile_pool(name="qw", bufs=2) as qw,
                tc.tile_pool(name="kvac", bufs=1, space="PSUM") as kvacp,
                tc.tile_pool(name="kvo", bufs=4, space="PSUM") as kvop,
                tc.tile_pool(name="qps", bufs=2, space="PSUM") as qpool,
            ):
                wkvh = [wkvp.tile([128, 2, 2 * C], F8, name=f"wkvh{j}",
                                  tag=f"wkvh{j}") for j in range(NJ)]
                wkvl = [wkvp.tile([128, 2, 2 * C], F8, name=f"wkvl{j}",
                                  tag=f"wkvl{j}") for j in range(NJ)]
                dmaeng = [nc.gpsimd, nc.sync, nc.scalar]
                # st0 x halves go out first so the first q chain (which only
                # needs wq + x) can start early; wq interleaves with them,
                # wkv rides behind (not needed until the first k/v block),
                # bulk broadcast consts last.
                # head schedule: the first q DR chain consumes (wqh, xah)
                # k-tile pairs in order j=0,1,2 then (wql, xah) then
                # (wqh, xal); chunked xa loads + j-ordered weight loads let
                # the PE start ~1.5us in instead of waiting for full tiles
                xa0h = xpool.tile([128, NCT, ST], F8, tag="xah")
                xa0l = xpool.tile([128, NCT, ST], F8, tag="xal")
                x0rh = xpk8h[0].rearrange("p (a t) -> p a t", t=ST)
                x0rl = xpk8l[0].rearrange("p (a t) -> p a t", t=ST)
                ropa0 = sa.tile([128, NB, 2, D], FP, tag="ropa")
                cT0 = sa.tile([128, ST], FP, tag="cT")
                sT0 = sa.tile([128, ST], FP, tag="sT")
                nc.sync.dma_start(out=wqh[0], in_=wq8h[0])
                nc.scalar.dma_start(out=wqh[1], in_=wq8h[1])
                nc.scalar.dma_start(out=wqh[2], in_=wq8h[2])
                nc.gpsimd.dma_start(out=wql[0], in_=wq8l[0])
                nc.gpsimd.dma_start(out=wql[1], in_=wq8l[1])
                nc.gpsimd.dma_start(out=wql[2], in_=wq8l[2])
                nc.sync.dma_start(out=xa0h[:, 0:2, :], in_=x0rh[:, 0:2, :])
                nc.sync.dma_start(out=xa0h[:, 2:4, :], in_=x0rh[:, 2:4, :])
                nc.scalar.dma_start(out=xa0h[:, 4:6, :], in_=x0rh[:, 4:6, :])
                for jc in range(NJ):
                    eng = nc.scalar if jc < 2 else nc.sync
                    eng.dma_start(out=xa0l[:, 2 * jc : 2 * jc + 2, :],
                                  in_=x0rl[:, 2 * jc : 2 * jc + 2, :])
                nc.gpsimd.dma_start(out=cT0, in_=cosT2[:, 0:ST])
                nc.gpsimd.dma_start(out=sT0, in_=sinT2[:, 0:ST])
                nc.gpsimd.dma_start(out=ropa0, in_=rope_pk[0])
                nc.sync.dma_start(out=wkvh[0], in_=wkv8h[0])
                nc.sync.dma_start(out=wkvh[1], in_=wkv8h[1])
                nc.scalar.dma_start(out=wkvh[2], in_=wkv8h[2])
                nc.sync.dma_start(out=wkvl[0], in_=wkv8l[0])
                nc.sync.dma_start(out=wkvl[1], in_=wkv8l[1])
                nc.scalar.dma_start(out=wkvl[2], in_=wkv8l[2])

                kvac = [
                    kvacp.tile([128, 512], FP, tag=f"kvac{i}", name=f"kvac{i}")
                    for i in range(2)
                ]
                # PE pstate warmup: the tensor engine runs at half speed for
                # its first ~3us of busy time. Burn the ramp on dummy zero
                # matmuls while the head DMAs are still in flight, so the
                # first real q chains run at full clock.
                wz = wk.tile([128, 384], BF, tag="warmz")
                nc.vector.tensor_copy(wz, zf[:, 0, :].rearrange(
                    "p (a b) -> p (a b)", a=1) if False else zf[:, 0:3, :]
                    .rearrange("p a b -> p (a b)")[:, 0:384])
                wps = kvop.tile([128, 384], FP, tag="kv_mm", name="warmps")
                for wi in range(6):
                    _mmb(nc, wps, wz[:, 0:128], wz, wi == 0, wi == 5)
                pending_kv = None

                def emit_kv_half(args, half):
                    ek, vnx, first, last = args
                    for pr in range(3 * half, 3 * half + 3):
                        _mmb(
                            nc,
                            kvac[half][:, (pr % 3) * 130 : (pr % 3) * 130 + 130],
                            ek[:, pr * 128 : (pr + 1) * 128],
                            vnx[:, pr, :],
                            first and pr % 3 == 0, last,
                        )

                def emit_kv(args):
                    emit_kv_half(args, 0)
                    emit_kv_half(args, 1)

                # ---- kvext / ksum-broadcast assembly (bf16), per bank,
                # emitted immediately after that bank's last kv chain stops
                # so the copies run ahead of the remaining phase-1 vector
                # stragglers. ksum broadcast: one strided ACT copy per
                # (bank, head-half) covers 3 pairs; ACT and DVE (unlike
                # gpsimd) may read PSUM.
                def assemble_bank(i):
                    acc = kvac[i]
                    if vb_zero:
                        # kv quadrants are plain strided copies
                        q0 = acc[0:64, 0:1]
                        nc.vector.tensor_copy(
                            kvx2[i][0:64, :, 0:64],
                            bass.AP(tensor=q0.tensor, offset=q0.offset,
                                    ap=[q0.ap[0], [130, 3], [1, 64]]))
                        q1 = acc[64:128, 64:65]
                        nc.vector.tensor_copy(
                            kvx2[i][64:128, :, 64:128],
                            bass.AP(tensor=q1.tensor, offset=q1.offset,
                                    ap=[q1.ap[0], [130, 3], [1, 64]]))
                    else:
                        for j in range(3):
                            off = j * 130
                            pr = 3 * i + j
                            h0, h1 = 2 * pr, 2 * pr + 1
                            nc.vector.scalar_tensor_tensor(
                                out=kvx2[i][0:64, j, 0:64],
                                in0=vbb[0:64, h0 * D : (h0 + 1) * D],
                                scalar=acc[0:64, off + 128 : off + 129],
                                in1=acc[0:64, off + 0 : off + 64],
                                op0=mybir.AluOpType.mult,
                                op1=mybir.AluOpType.add,
                            )
                            nc.vector.scalar_tensor_tensor(
                                out=kvx2[i][64:128, j, 64:128],
                                in0=vbb[64:128, h1 * D : (h1 + 1) * D],
                                scalar=acc[64:128, off + 129 : off + 130],
                                in1=acc[64:128, off + 64 : off + 128],
                                op0=mybir.AluOpType.mult,
                                op1=mybir.AluOpType.add,
                            )
                    b0 = acc[0:64, 128:129]
                    nc.scalar.copy(
                        ksdb[i][0:64, :, 0:64],
                        bass.AP(tensor=b0.tensor, offset=b0.offset,
                                ap=[b0.ap[0], [130, 3], [0, 64]]))
                    b1 = acc[64:128, 129:130]
                    nc.scalar.copy(
                        ksdb[i][64:128, :, 64:128],
                        bass.AP(tensor=b1.tensor, offset=b1.offset,
                                ap=[b1.ap[0], [130, 3], [0, 64]]))

                def do_q(st, tsl, xah, xal, cT, sT):
                    # --- q: feature-major fp8 split-3 DoubleRow, RoPE, exp ---
                    nonlocal pending_kv
                    def q_chain_part(qps, fsl, xah, xal, terms, start, stop):
                        n = 0
                        for (w, x) in terms:
                            for j in range(NJ):
                                nc.tensor.matmul(
                                    qps, w[j][:, :, fsl],
                                    x[:, 2 * j : 2 * j + 2, :],
                                    start=start and n == 0,
                                    stop=stop and n == NJ * len(terms) - 1,
                                    perf_mode=DRM,
                                )
                                n += 1

                    qps_pre = {}
                    if st == 0:
                        # head: the xal chunks land ~5us in (DMA latency);
                        # run pairs 0/1's (wh,xh)+(wl,xh) terms first so the
                        # PE has work while xal is in flight
                        for fq in (0, 1):
                            qps_pre[fq] = qpool.tile([128, ST], FP, tag="q",
                                                     name=f"qpre{fq}")
                            q_chain_part(qps_pre[fq],
                                         slice(fq * 128, (fq + 1) * 128),
                                         xah, xal,
                                         ((wqh, xah), (wql, xah)),
                                         True, False)
                    for fq in range(NPAIR):
                        fsl = slice(fq * 128, (fq + 1) * 128)
                        if fq in qps_pre:
                            qps = qps_pre[fq]
                            q_chain_part(qps, fsl, xah, xal,
                                         ((wqh, xal),), False, True)
                        else:
                            qps = qpool.tile([128, ST], FP, tag="q")
                            q_chain_part(qps, fsl, xah, xal,
                                         ((wqh, xah), (wql, xah),
                                          (wqh, xal)), True, True)
                        if fq == 0 and pending_kv is not None:
                            # previous block's kv mms ride behind the first q
                            # chain so the PE isn't waiting on vnx
                            emit_kv(pending_kv)
                            pending_kv = None
                        qs = qw.tile([128, ST], FP, tag="qs")
                        nc.vector.stream_shuffle(qs, qps, PAIRSWAP32)
                        # gpsimd cannot touch PSUM on hw; qps reads stay on DVE
                        t1q = qw.tile([128, ST], FP, tag="qt1")
                        nc.vector.tensor_mul(t1q, qps, cT)
                        t2q = qw.tile([128, ST], FP, tag="qt2")
                        nc.gpsimd.tensor_mul(t2q, qs, sT)
                        eqin = qw.tile([128, ST], FP, tag="eqin")
                        nc.gpsimd.tensor_add(eqin, t1q, t2q)
                        nc.scalar.activation(
                            out=eqs[fq][:, tsl],
                            in_=eqin,
                            func=mybir.ActivationFunctionType.Exp,
                            bias=qb_sb[:, fq : fq + 1],
                            scale=1.0 / WS,
                        )

                def kv_mm_chain(ps, xah, xal, bsl, csl):
                    # fp8 split-3 DoubleRow chain: out[bsl-tokens, csl-cols]
                    n = 0
                    for (x, w) in ((xah, wkvh), (xal, wkvh), (xah, wkvl)):
                        for j in range(NJ):
                            nc.tensor.matmul(
                                ps, x[:, 2 * j : 2 * j + 2, bsl],
                                w[j][:, :, csl],
                                start=n == 0, stop=n == 3 * NJ - 1,
                                perf_mode=DRM,
                            )
                            n += 1

                def do_blocks(st, tsl, xah, xal, ropa):
                    nonlocal pending_kv
                    for b in range(NB):
                        if pending_kv is not None:
                            emit_kv(pending_kv)
                            pending_kv = None
                        ek = ekpool.tile([128, C], BF, tag="ek")
                        bsl = slice(b * 128, (b + 1) * 128)
                        # --- k: wkv cols [0, 768), 2 tiles of 384 ---
                        for kt in range(2):
                            kps = kvop.tile([128, 384], FP, tag="kv_mm")
                            kv_mm_chain(kps, xah, xal, bsl,
                                        slice(kt * 384, (kt + 1) * 384))
                            cosb = bass.AP(
                                tensor=ropa.tensor,
                                offset=ropa.offset + (b * 2) * D,
                                ap=[ropa.ap[0], [0, 6], [1, D]],
                            )
                            # PSUM -> SBUF copy so the gpsimd rope ops can
                            # read it (gpsimd cannot touch PSUM on hw)
                            ksb = wk.tile([128, 384], FP, tag="ksb")
                            nc.scalar.copy(ksb, kps)
                            t1 = wk.tile([128, 384], FP, tag="t1")
                            nc.gpsimd.tensor_mul(t1, ksb, cosb)
                            t2 = wk.tile([128, 384], FP, tag="t2")
                            ksb3 = ksb.rearrange("p (x two) -> p x two", two=2)
                            t23 = t2.rearrange("p (x two) -> p x two", two=2)
                            sin_e = bass.AP(
                                tensor=ropa.tensor,
                                offset=ropa.offset + (b * 2 + 1) * D,
                                ap=[ropa.ap[0], [0, 6], [2, 32]],
                            )
                            sin_o = bass.AP(
                                tensor=ropa.tensor,
                                offset=ropa.offset + (b * 2 + 1) * D + 1,
                                ap=[ropa.ap[0], [0, 6], [2, 32]],
                            )
                            nc.gpsimd.tensor_mul(t23[:, :, 0], ksb3[:, :, 1], sin_e)
                            nc.gpsimd.tensor_mul(t23[:, :, 1], ksb3[:, :, 0], sin_o)
                            krin = wk.tile([128, 384], FP, tag="krin")
                            nc.gpsimd.tensor_add(krin, t1, t2)
                            nc.scalar.activation(
                                out=ek[:, kt * 384 : (kt + 1) * 384],
                                in_=krin,
                                func=mybir.ActivationFunctionType.Exp,
                                scale=1.0 / WS,
                            )
                        # --- per-half rowsum + v so half 0's kv inputs are
                        # ready before half 1's v matmuls finish ---
                        sk = sa.tile([128, H], FP, tag="sk")
                        ski = sa.tile([128, H], FP, tag="ski")
                        vnx = vnxpool.tile([128, NPAIR, 130], BF, tag="vnx")
                        last_blk = st == NST - 1 and b == NB - 1
                        args = (ek, vnx, st == 0 and b == 0, last_blk)
                        for vt in range(2):
                            hsl = slice(vt * 6, vt * 6 + 6)
                            nc.vector.reduce_sum(
                                sk[:, hsl],
                                ek[:, vt * 384 : (vt + 1) * 384].rearrange(
                                    "p (h d) -> p h d", d=D),
                                axis=mybir.AxisListType.X,
                            )
                            nc.vector.reciprocal(ski[:, hsl], sk[:, hsl])
                            nc.vector.tensor_copy(
                                vnx[:, 3 * vt : 3 * vt + 3, 128:130],
                                ski[:, hsl].rearrange("p (a b) -> p a b", b=2),
                            )
                            vps = kvop.tile([128, 384], FP, tag="kv_mm")
                            kv_mm_chain(vps, xah, xal, bsl,
                                        slice(C + vt * 384, C + (vt + 1) * 384))
                            vps4 = vps.rearrange("p (pr q d) -> p pr q d", q=2, d=D)
                            skib = bass.AP(
                                tensor=ski.tensor,
                                offset=ski.offset + vt * 6,
                                ap=[ski.ap[0], [2, 3], [1, 2], [0, D]],
                            )
                            if last_blk:
                                # split the serial DVE tail: stage vps via
                                # ACT (parallel with the DVE rowsum) and
                                # multiply on Pool
                                vsb = wk.tile([128, 384], FP, tag="vsb")
                                nc.scalar.copy(vsb, vps)
                                nc.gpsimd.tensor_mul(
                                    vnx[:, 3 * vt : 3 * vt + 3, 0:128]
                                    .rearrange("p pr (q d) -> p pr q d", d=D),
                                    vsb.rearrange(
                                        "p (pr q d) -> p pr q d", q=2, d=D),
                                    skib,
                                )
                            else:
                                nc.vector.tensor_mul(
                                    vnx[:, 3 * vt : 3 * vt + 3, 0:128]
                                    .rearrange("p pr (q d) -> p pr q d", d=D),
                                    vps4,
                                    skib,
                                )
                            if last_blk:
                                # flush this bank's kv chain immediately so
                                # its assembly can start while the other
                                # half's v path is still in flight
                                emit_kv_half(args, vt)
                                assemble_bank(vt)
                        pending_kv = None if last_blk else args

                for st in range(NST):
                    tsl = slice(st * ST, (st + 1) * ST)
                    if st == 0:
                        xah, xal, ropa, cT, sT = xa0h, xa0l, ropa0, cT0, sT0
                    else:
                        xah = xpool.tile([128, NCT, ST], F8, tag="xah")
                        nc.sync.dma_start(
                            out=xah,
                            in_=xpk8h[st].rearrange("p (a t) -> p a t", t=ST))
                        xal = xpool.tile([128, NCT, ST], F8, tag="xal")
                        nc.sync.dma_start(
                            out=xal,
                            in_=xpk8l[st].rearrange("p (a t) -> p a t", t=ST))
                        ropa = sa.tile([128, NB, 2, D], FP, tag="ropa")
                        nc.scalar.dma_start(out=ropa, in_=rope_pk[st])
                        cT = sa.tile([128, ST], FP, tag="cT")
                        nc.scalar.dma_start(out=cT, in_=cosT2[:, tsl])
                        sT = sa.tile([128, ST], FP, tag="sT")
                        nc.scalar.dma_start(out=sT, in_=sinT2[:, tsl])
                    if st == 1:
                        for ct in range(NCT):
                            dmaeng[1 + ct % 2].dma_start(
                                out=pwgs[ct],
                                in_=pwgT[ct * 128 : (ct + 1) * 128, :])
                        nc.sync.dma_start(
                            out=vbb,
                            in_=bass.AP(tensor=vb, offset=0,
                                        ap=[[0, 128], [1, C]]))
                        nc.sync.dma_start(
                            out=cb_sb,
                            in_=bass.AP(tensor=constb, offset=0,
                                        ap=[[0, 128], [1, C]]))
                        nc.sync.dma_start(
                            out=wsb,
                            in_=bass.AP(tensor=wsn2, offset=0,
                                        ap=[[0, 128], [1, C]]))
                    if st == NST - 1:
                        # last supertile: k/v/kv first so the kv assembly
                        # (which gates phase 2) overlaps the q chains on PE
                        do_blocks(st, tsl, xah, xal, ropa)
                        do_q(st, tsl, xah, xal, cT, sT)
                    else:
                        do_q(st, tsl, xah, xal, cT, sT)
                        do_blocks(st, tsl, xah, xal, ropa)

                if pending_kv is not None:
                    emit_kv(pending_kv)
                    pending_kv = None

            # ============ PHASE 2: attn -> LN -> proj ============
            with (
                tc.tile_pool(name="attp", bufs=3) as attpool,
                tc.tile_pool(name="o2p", bufs=6) as o2pool,
                tc.tile_pool(name="zrp", bufs=2) as zrpool,
                tc.tile_pool(name="rows", bufs=2) as rows,
                tc.tile_pool(name="colsb", bufs=4) as colsb,
                tc.tile_pool(name="w2p", bufs=2) as w2pool,
                tc.tile_pool(name="otp", bufs=4) as opool,
                tc.tile_pool(name="yps", bufs=2, space="PSUM") as ypool,
                tc.tile_pool(name="zps", bufs=2, space="PSUM") as zpool,
                tc.tile_pool(name="pps", bufs=3, space="PSUM") as ppool,
                tc.tile_pool(name="smallps", bufs=1, space="PSUM") as smallp,
            ):
                def proj_chain(att, b, jt, lo=None, n=None):
                    pps = ppool.tile([128, 512], FP, tag="proj")
                    bsl = slice(b * 128, (b + 1) * 128)
                    if lo is None:
                        lo, n = (0, 512) if jt == 0 else (512, 258)
                    for ct in range(NCT):
                        _mm(nc, pps[:, 0:n],
                            att[ct][:, bsl],
                            pwgs[ct][:, lo : lo + n],
                            ct == 0, ct == NCT - 1)
                    return pps

                for st in range(NST):
                    tsl = slice(st * ST, (st + 1) * ST)
                    att = [attpool.tile([128, ST], FR, tag=f"att{ct}",
                                        name=f"att{ct}")
                           for ct in range(NCT)]
                    s2args = []
                    # block 0's wide proj chain interleaves into the pair
                    # loop: its ct-k matmul only needs pair k's att, so it
                    # fills the PE slots where ya/zb wait on PSUM recycling
                    pps0_b0 = ppool.tile([128, 512], FP, tag="proj")
                    for fq in range(NPAIR):
                        ya = ypool.tile([128, ST], FP, tag="yps")
                        _mmb(nc, ya, kvx2[fq // 3][:, fq % 3, :],
                             eqs[fq][:, tsl], True, True)
                        zb = zpool.tile([128, ST], FP, tag="zps")
                        _mmb(nc, zb, ksdb[fq // 3][:, fq % 3, :],
                             eqs[fq][:, tsl], True, True)
                        zr = zrpool.tile([128, ST], FP, tag="zr")
                        nc.vector.reciprocal_approx_fast(out=zr, in_=zb)
                        if fq >= 2:
                            ct = fq - 2
                            _mm(nc, pps0_b0, att[ct][:, 0:128],
                                pwgs[ct][:, 0:512], ct == 0, False)
                        if fq == NPAIR - 1:
                            # critical last pair: DVE reads ya from PSUM
                            # directly, skipping two engine crossings
                            nc.vector.tensor_mul(att[fq], ya, zr)
                        else:
                            # gpsimd cannot read PSUM: stage ya via ACT
                            ya_sb = zrpool.tile([128, ST], FP, tag="ya_sb")
                            nc.scalar.copy(ya_sb, ya)
                            nc.gpsimd.tensor_mul(att[fq], ya_sb, zr)
                    for ct in (4, 5):
                        _mm(nc, pps0_b0, att[ct][:, 0:128],
                            pwgs[ct][:, 0:512], False, ct == NCT - 1)
                    # squares emitted after the pair loop so the ACT queue
                    # drains the ya copies first (they gate PSUM recycling)
                    o2s = []
                    for fq in range(NPAIR):
                        o2 = o2pool.tile([128, ST], BF, tag="o2")
                        nc.scalar.activation(
                            out=o2, in_=att[fq].bitcast(FP),
                            func=mybir.ActivationFunctionType.Square,
                        )
                        o2s.append(o2)
                    for g in range(2):
                        ga = o2pool.tile([128, ST], BF, tag="o2ga")
                        eng = nc.vector if g == 1 else nc.gpsimd
                        eng.tensor_add(ga, o2s[3 * g], o2s[3 * g + 1])
                        gs = o2pool.tile([128, ST], BF, tag="o2g")
                        eng.tensor_add(gs, ga, o2s[3 * g + 2])
                        s2args.append(gs)

                    def block_tail(st, b, pps0, pps1, s2cs):
                        # LN stats as [128,1] columns; mu rides in pps1 col
                        # 256 (pwg ones-column is 1/C), s2c already has +eps.
                        # var = s2c - mu^2 computed as sqrt(s2c + (-mu^2))
                        # with the negated square as an ACT bias column.
                        mu = pps1[:, 256:257]
                        nmsq = colsb.tile([128, 1], FP, tag="nmsq")
                        nc.vector.tensor_scalar(
                            out=nmsq, in0=mu, scalar1=-1.0, scalar2=mu,
                            op0=mybir.AluOpType.mult,
                            op1=mybir.AluOpType.mult)
                        sd = colsb.tile([128, 1], FP, tag="sd")
                        nc.scalar.activation(
                            out=sd, in_=s2cs[b],
                            func=mybir.ActivationFunctionType.Sqrt,
                            bias=nmsq)
                        rc = colsb.tile([128, 1], FP, tag="rc")
                        nc.vector.reciprocal(rc, sd)
                        # out = rc*(P + mu*(-wsum)) + const; the X ops (DVE,
                        # the only engine here that may read PSUM) read and
                        # release the proj PSUM banks early.
                        x0 = w2pool.tile([128, 512], FP, tag="x0")
                        x1 = w2pool.tile([128, 256], FP, tag="x1")
                        nc.vector.scalar_tensor_tensor(
                            out=x0, in0=wsb[:, 0:512], scalar=mu, in1=pps0,
                            op0=mybir.AluOpType.mult,
                            op1=mybir.AluOpType.add)
                        nc.vector.scalar_tensor_tensor(
                            out=x1, in0=wsb[:, 512:768], scalar=mu,
                            in1=pps1[:, 0:256],
                            op0=mybir.AluOpType.mult,
                            op1=mybir.AluOpType.add)
                        # Pool supports only plain tensor-tensor ops: scale
                        # by rc via a free-broadcast AP, then add the const
                        ot0 = opool.tile([128, 512], FP, tag="ot")
                        ot1 = opool.tile([128, 512], FP, tag="ot")
                        rcb = bass.AP(tensor=rc.tensor, offset=rc.offset,
                                      ap=[rc.ap[0], [0, 512]])
                        t0 = w2pool.tile([128, 512], FP, tag="t0")
                        nc.gpsimd.tensor_mul(t0, x0, rcb)
                        nc.gpsimd.tensor_add(ot0, t0, cb_sb[:, 0:512])
                        rcb2 = bass.AP(tensor=rc.tensor, offset=rc.offset,
                                       ap=[rc.ap[0], [0, 256]])
                        t1 = w2pool.tile([128, 256], FP, tag="t1")
                        nc.gpsimd.tensor_mul(t1, x1, rcb2)
                        nc.gpsimd.tensor_add(ot1[:, 0:256], t1,
                                             cb_sb[:, 512:768])
                        row0 = st * ST + b * 128
                        nc.sync.dma_start(
                            out=out[row0 : row0 + 128, 0:512], in_=ot0)
                        nc.sync.dma_start(
                            out=out[row0 : row0 + 128, 512:768],
                            in_=ot1[:, 0:256])

                    # block0's wide chain was emitted inside the pair loop;
                    # the s2 chain follows (its inputs are ready by then)
                    s2p = smallp.tile([1, ST], FP, tag="sm", name="s2p")
                    for g in range(2):
                        _mmb(nc, s2p, ones128b, s2args[g], g == 0, g == 1)
                    # att rides at su=SU scale (v path is computed from
                    # SU-scaled weights); LN is scale-invariant except for
                    # eps, which must scale by su^2.
                    s2row = rows.tile([1, ST], FP, tag="s2row")
                    nc.vector.tensor_scalar(
                        out=s2row, in0=s2p, scalar1=1.0 / C,
                        scalar2=EPS_LN * SU * SU,
                        op0=mybir.AluOpType.mult, op1=mybir.AluOpType.add)
                    s2ct = smallp.tile([128, NB], FP, tag="sm", name="s2ct")
                    for b in range(NB):
                        # fp32 (not f32r): free-size-1 f32r matmuls fail the
                        # hw ISA check
                        nc.tensor.matmul(
                            s2ct[:, b : b + 1],
                            s2row[:, b * 128 : (b + 1) * 128], one11,
                            start=True, stop=True)
                    s2cs = [s2ct[:, b : b + 1] for b in range(NB)]
                    pps1_b0 = proj_chain(att, 0, 1)
                    block_tail(st, 0, pps0_b0, pps1_b0, s2cs)
                    for b in range(1, NB):
                        if st == NST - 1 and b == NB - 1:
                            # last block: narrow (stats) chain first, wide
                            # chain as two 256-col PSUM chains so each
                            # half's x->t->ot->store pipeline starts as soon
                            # as its half stops accumulating
                            pps1 = proj_chain(att, b, 1)
                            pps0a = proj_chain(att, b, 0, lo=0, n=256)
                            pps0b = proj_chain(att, b, 0, lo=256, n=256)
                            mu = pps1[:, 256:257]
                            nmsq = colsb.tile([128, 1], FP, tag="nmsq")
                            nc.vector.tensor_scalar(
                                out=nmsq, in0=mu, scalar1=-1.0, scalar2=mu,
                                op0=mybir.AluOpType.mult,
                                op1=mybir.AluOpType.mult)
                            sd = colsb.tile([128, 1], FP, tag="sd")
                            nc.scalar.activation(
                                out=sd, in_=s2cs[b],
                                func=mybir.ActivationFunctionType.Sqrt,
                                bias=nmsq)
                            rc = colsb.tile([128, 1], FP, tag="rc")
                            nc.vector.reciprocal(rc, sd)
                            row0 = st * ST + b * 128
                            rcb = bass.AP(
                                tensor=rc.tensor, offset=rc.offset,
                                ap=[rc.ap[0], [0, 256]])
                            # narrow 256 first (its PSUM stopped earliest)
                            x1 = w2pool.tile([128, 256], FP, tag="x1")
                            nc.vector.scalar_tensor_tensor(
                                out=x1, in0=wsb[:, 512:768], scalar=mu,
                                in1=pps1[:, 0:256],
                                op0=mybir.AluOpType.mult,
                                op1=mybir.AluOpType.add)
                            t1 = w2pool.tile([128, 256], FP, tag="t1")
                            nc.gpsimd.tensor_mul(t1, x1, rcb)
                            ot1 = opool.tile([128, 256], FP, tag="ot1t")
                            nc.gpsimd.tensor_add(ot1, t1, cb_sb[:, 512:768])
                            nc.scalar.dma_start(
                                out=out[row0 : row0 + 128, 512:768], in_=ot1)
                            for h, pph in ((0, pps0a), (1, pps0b)):
                                csl = slice(h * 256, (h + 1) * 256)
                                xh = w2pool.tile([128, 256], FP, tag=f"xh{h}")
                                nc.vector.scalar_tensor_tensor(
                                    out=xh, in0=wsb[:, csl], scalar=mu,
                                    in1=pph[:, 0:256],
                                    op0=mybir.AluOpType.mult,
                                    op1=mybir.AluOpType.add)
                                th = w2pool.tile([128, 256], FP, tag=f"th{h}")
                                oth = opool.tile([128, 256], FP,
                                                 tag=f"oth{h}")
                                # half 0 on Pool, half 1 on DVE so the two
                                # post-rc chains drain in parallel
                                veng = nc.gpsimd if h == 0 else nc.vector
                                veng.tensor_mul(th, xh, rcb)
                                veng.tensor_add(oth, th, cb_sb[:, csl])
                                eng = nc.scalar if h == 0 else nc.sync
                                eng.dma_start(
                                    out=out[row0 : row0 + 128, csl], in_=oth)
                        else:
                            pps0 = proj_chain(att, b, 0)
                            pps1 = proj_chain(att, b, 1)
                            block_tail(st, b, pps0, pps1, s2cs)
    nc.finalize()
    return nc


@functools.lru_cache(maxsize=2)
def _get_nc(vb_zero=True):
    return build_nc(vb_zero)


def _split8(a):
    import ml_dtypes
    f8 = ml_dtypes.float8_e4m3
    h = np.asarray(a, np.float32).astype(f8)
    l = (np.asarray(a, np.float32) - h.astype(np.float32)).astype(f8)
    return h, l


def _pack_dr(a):
    # [C(contraction), F] -> [NJ, 128, 2, F] with k-tile pairs interleaved
    F = a.shape[1]
    return np.ascontiguousarray(
        a.reshape(NJ, 2, 128, F).transpose(0, 2, 1, 3))


def _prep_shared(qkv_w, q_bias, v_bias, norm_g, norm_b, proj_w, proj_b, rope):
    f = np.float32
    W = np.asarray(qkv_w, f)
    wqT = np.ascontiguousarray(W[0:C].T) * WS
    wkvT = np.ascontiguousarray(W[C:].T)
    wkvT[:, 0:C] *= WS     # k columns
    wkvT[:, C:] *= SU      # v columns (att rides at SU scale)
    wqTh, wqTl = _split8(wqT)
    wkvTh, wkvTl = _split8(wkvT)

    s = np.asarray(rope, f)[:, :D]
    c = np.asarray(rope, f)[:, D:]
    cos_tm = np.ones((N, D), f)
    sin_tm = np.zeros((N, D), f)
    cos_tm[NPT:] = c
    sin_tm[NPT:, 0::2] = -s[:, 0::2]
    sin_tm[NPT:, 1::2] = s[:, 1::2]
    cosT2 = np.ascontiguousarray(np.tile(cos_tm.T, (2, 1)))
    sinT2 = np.ascontiguousarray(np.tile(sin_tm.T, (2, 1)))
    # rope_pk[st, p, b, 0/1, d] = cos/sin_tm[st*512 + b*128 + p, d]
    rope_pk = np.ascontiguousarray(
        np.stack([cos_tm, sin_tm], axis=1)           # [N, 2, D]
        .reshape(NST, NB, 128, 2, D)
        .transpose(0, 2, 1, 3, 4)                    # [NST, 128, NB, 2, D]
    )

    g = np.asarray(norm_g, f)
    bb = np.asarray(norm_b, f)
    P = np.asarray(proj_w, f)
    pwg = (P * g[None, :]).T                         # [C, C]
    pwgT = np.zeros((C, 770), f)
    pwgT[:, 0:C] = pwg
    pwgT[:, C] = 1.0 / C
    wsn2 = np.ascontiguousarray(-pwg.sum(axis=0))
    constb = np.ascontiguousarray(np.asarray(proj_b, f) + P @ bb)
    return dict(
        wq8h=_pack_dr(wqTh), wq8l=_pack_dr(wqTl),
        wkv8h=_pack_dr(wkvTh), wkv8l=_pack_dr(wkvTl),
        pwgT=np.ascontiguousarray(pwgT), cosT2=cosT2, sinT2=sinT2, rope_pk=rope_pk,
        qb=np.ascontiguousarray(np.asarray(q_bias, f)),
        # v rides at SU scale on-chip (kv accum / att / LN-invariance)
        vb=np.ascontiguousarray(np.asarray(v_bias, f) * SU),
        wsn2=wsn2, constb=constb,
    )


def kernel(x, rope, qkv_w, q_bias, v_bias, norm_g, norm_b, proj_w, proj_b,
           num_heads, num_prefix_tokens, _trace=False):
    assert int(num_heads) == H and int(num_prefix_tokens) == NPT
    x = np.asarray(x, np.float32)
    assert x.shape == (B, N, C)
    vbz = bool(np.all(np.asarray(v_bias) == 0.0))
    shared = _prep_shared(qkv_w, q_bias, v_bias, norm_g, norm_b, proj_w,
                          proj_b, rope)
    xh8, xl8 = _split8(x)
    in_maps = []
    for bi in range(B):
        m = dict(shared)
        for key, xs in (("xpk8h", xh8), ("xpk8l", xl8)):
            xt = xs[bi].T  # [C, N] fp8
            m[key] = np.ascontiguousarray(
                xt.reshape(NCT, 128, NST, ST).transpose(2, 1, 0, 3)
                .reshape(NST, 128, NCT * ST)
            )
        in_maps.append(m)
    nc = _get_nc(vbz)
    res = run_bass_kernel_spmd(nc, in_maps, core_ids=list(range(B)),
                               trace=_trace)
    out = np.stack([res.results[bi]["out"] for bi in range(B)], axis=0)
    if _trace:
        kernel.last_results = res
    return out

